# revision 24
# baseline (speedup 1.0000x reference)
"""Trainium2 Bass kernel for the CNN-TRX few-shot attention head.

Sharding: data-parallel over the 200 queries (25 per NeuronCore); support set
and weights replicated per core. All matmuls in bf16 with fp32 PSUM:

  1. Frame projection in transposed layout: f_T[d, frame] for all 6 weight
     blocks (k_w/v_w x 3 tuple positions); biases folded via an augmented
     ones-row of X.
  2. Tuple gather (C(8,3)=56 frame triples) as 2-stage DVE column adds.
  3. LayerNorm of K projections column-wise: stats via ones-matmuls, Rsqrt on
     ACT, gpsimd partition-broadcast, two DVE passes.
  4. scoresT = s_k_pad^T q_k with supports sorted by class and class blocks
     padded to 128 rows; exp via ACT (no max-subtract: LN'd scores are O(1),
     Cauchy-Schwarz bounds |score| <= 34 so exp stays finite in fp32).
  5. Per-class prototypes in T-layout; distance terms ||q_v||^2, <q_v,P>,
     ||P||^2, sum(exp) via ones-matmul column reductions; final combine on
     single-partition rows; logits = -sum_a dist / 56.
"""

import math
from itertools import combinations

import ml_dtypes
import numpy as np

SEQ = 8
IN_DIM = 2048
OUT_DIM = 1152
TSS = 3
WAY = 5
N_SUPPORT = 25
N_QUERIES = 200
PE_SCALE = 0.1
LN_EPS = 1e-5
T = 56
N_CORES = 8
NQL = N_QUERIES // N_CORES      # queries per core
G_Q = 5                         # queries per inner group
N_GROUPS = NQL // G_Q
C = G_Q * T                     # score columns per group (280)
KPAD = 2176                     # 17 * 128 (2048 data + ones row + zero pad)
NKCH = KPAD // 128
NDCH = OUT_DIM // 128           # 9
NMB = 6 * OUT_DIM // 128        # 54 projection column blocks
NX = SEQ * 2 * N_SUPPORT        # 400 frame columns per core
PAIRS = [(t0, t1) for t0 in range(SEQ - 2) for t1 in range(t0 + 1, SEQ - 1)]
LN_CHUNK = 448                  # LayerNorm column chunk (PSUM free-dim <= 512)
BF16 = ml_dtypes.bfloat16

_CACHE = {}


def _pos_encoding():
    pos = np.arange(SEQ, dtype=np.float32)[:, None]
    div = np.exp(np.arange(0, IN_DIM, 2, dtype=np.float32) * -(math.log(10000.0) / IN_DIM))
    pe = np.zeros((SEQ, IN_DIM), dtype=np.float32)
    pe[:, 0::2] = np.sin(pos * div) * PE_SCALE
    pe[:, 1::2] = np.cos(pos * div) * PE_SCALE
    return pe


def _class_layout(counts):
    offs, off = [], 0
    for c in range(WAY):
        offs.append(off)
        off += ((counts[c] * T + 127) // 128) * 128
    return offs, off


def _build_kernel(counts, trivial_gb):
    import concourse.mybir as mybir
    import concourse.tile as tile
    from concourse import bacc
    from concourse.masks import make_identity

    f32 = mybir.dt.float32
    bf16 = mybir.dt.bfloat16
    AF = mybir.ActivationFunctionType
    ALU = mybir.AluOpType
    offs, nb_pad = _class_layout(counts)
    nwch = nb_pad // 128
    inv_sqrt = 1.0 / math.sqrt(OUT_DIM)

    nc = bacc.Bacc("TRN2", target_bir_lowering=False, debug=False,
                   enable_asserts=False, num_devices=N_CORES)

    x_d = nc.dram_tensor("x", [128, NKCH, NX], bf16, kind="ExternalInput").ap()
    w_d = nc.dram_tensor("w", [128, NMB, NKCH, 128], bf16, kind="ExternalInput").ap()
    g_d = nc.dram_tensor("lng", [128, NDCH], bf16, kind="ExternalInput").ap()
    b_d = nc.dram_tensor("lnb", [128, NDCH], bf16, kind="ExternalInput").ap()
    out_d = nc.dram_tensor("out", [NQL, WAY], f32, kind="ExternalOutput").ap()

    with tile.TileContext(nc) as tc:
        with tc.tile_pool(name="big", bufs=1) as big, \
             tc.tile_pool(name="small", bufs=1) as small:
            f_T = big.tile([128, 6 * NDCH, NX], bf16)       # frame projections, T-layout
            s_kT = big.tile([128, NDCH, nb_pad], bf16)      # LN'd support K, padded cols
            s_v = big.tile([128, nwch, OUT_DIM], bf16)      # support V, row-natural padded
            ones_sb = small.tile([128, 1], bf16)
            nc.vector.memset(ones_sb, 1.0)
            eps_sb = small.tile([1, 1], f32)
            nc.vector.memset(eps_sb, LN_EPS)
            g_sb = small.tile([128, NDCH], bf16)
            b_sb = small.tile([128, NDCH], bf16)
            nc.sync.dma_start(g_sb, g_d)
            nc.sync.dma_start(b_sb, b_d)
            logits5 = small.tile([WAY, NQL], f32)

            # ---------- Phase 1: frame projections ----------
            with tc.tile_pool(name="xt_pool", bufs=1) as xt_pool, \
                 tc.tile_pool(name="xw", bufs=3) as xw, \
                 tc.tile_pool(name="pp_proj", bufs=4, space="PSUM") as pp_proj:
                xt = xt_pool.tile([128, NKCH, NX], bf16)
                nc.sync.dma_start(xt, x_d)
                for m in range(NMB):
                    wm = xw.tile([128, NKCH, 128], bf16, tag="wslab")
                    nc.sync.dma_start(wm, w_d[:, m])
                    ps = pp_proj.tile([128, NX], f32, tag="projps")
                    for k in range(NKCH):
                        nc.tensor.matmul(ps, wm[:, k], xt[:, k],
                                         start=(k == 0), stop=(k == NKCH - 1))
                    nc.scalar.activation(f_T[:, m], ps, AF.Copy)

            # f_T blocks: m = kv*27 + j*NDCH + dd
            f6 = f_T.rearrange("p (kv j d) (i s) -> p kv j d i s", kv=2, j=TSS, s=SEQ)

            def gather(dst5, items0, n_items, pool):
                """dst5 [128, 2, NDCH, n_items, T] = tuple-gathered frame
                projections for the K and V paths. DVE ISA allows at most 3
                free AP dims, so K/V are separate ops."""
                isl = slice(items0, items0 + n_items)
                p2 = pool.tile([128, 2, NDCH, n_items, len(PAIRS)], bf16, tag="pairs")
                for kv in range(2):
                    pi = 0
                    for t0 in range(SEQ - 2):
                        run = SEQ - 2 - t0
                        a = f6[:, kv, 0, :, isl, t0:t0 + 1]
                        b = f6[:, kv, 1, :, isl, t0 + 1:t0 + 1 + run]
                        nc.vector.tensor_add(p2[:, kv, :, :, pi:pi + run],
                                             a.to_broadcast(b.shape), b)
                        pi += run
                    ai = 0
                    for pi, (t0, t1) in enumerate(PAIRS):
                        run = SEQ - 1 - t1
                        a = p2[:, kv, :, :, pi:pi + 1]
                        b = f6[:, kv, 2, :, isl, t1 + 1:t1 + 1 + run]
                        nc.vector.tensor_add(dst5[:, kv, :, :, ai:ai + run],
                                             a.to_broadcast(b.shape), b)
                        ai += run

            def col_ln(raw, cols, pool, psum_pool, out=None):
                """Column-wise LayerNorm of raw [128, NDCH, cols] (T-layout);
                in place unless `out` is given."""
                if out is None:
                    out = raw
                for c0 in range(0, cols, LN_CHUNK):
                    cw = min(LN_CHUNK, cols - c0)
                    r = raw[:, :, c0:c0 + cw]
                    o = out[:, :, c0:c0 + cw]
                    sq = pool.tile([128, NDCH, cw], bf16, tag="lnsq", name="lnsq", bufs=1)
                    nc.scalar.activation(sq, r, AF.Square)
                    ps_s = psum_pool.tile([1, cw], f32, tag="lnps", name="lnps")
                    ps_q = psum_pool.tile([1, cw], f32, tag="lnps", name="lnps")
                    for k in range(NDCH):
                        nc.tensor.matmul(ps_s, ones_sb, r[:, k],
                                         start=(k == 0), stop=(k == NDCH - 1))
                    for k in range(NDCH):
                        nc.tensor.matmul(ps_q, ones_sb, sq[:, k],
                                         start=(k == 0), stop=(k == NDCH - 1))
                    m_r = pool.tile([1, cw], f32, tag="lnm", name="lnm")
                    v_r = pool.tile([1, cw], f32, tag="lnv", name="lnv")
                    mm = pool.tile([1, cw], f32, tag="lnmm", name="lnmm")
                    nc.scalar.activation(m_r, ps_s, AF.Copy, scale=1.0 / OUT_DIM)
                    nc.scalar.activation(v_r, ps_q, AF.Copy, scale=1.0 / OUT_DIM)
                    nc.vector.tensor_mul(mm, m_r, m_r)
                    nc.vector.tensor_sub(v_r, v_r, mm)
                    nc.scalar.activation(v_r, v_r, AF.Sqrt, bias=eps_sb)
                    nc.vector.reciprocal(v_r, v_r)
                    m_b = pool.tile([128, cw], f32, tag="lnmb", name="lnmb", bufs=1)
                    a_b = pool.tile([128, cw], f32, tag="lnab", name="lnab", bufs=1)
                    nc.gpsimd.partition_broadcast(m_b, m_r)
                    nc.gpsimd.partition_broadcast(a_b, v_r)
                    mb3 = m_b[:, None, :].to_broadcast([128, NDCH, cw])
                    ab3 = a_b[:, None, :].to_broadcast([128, NDCH, cw])
                    nc.vector.tensor_sub(r, r, mb3)
                    nc.vector.tensor_mul(o, r, ab3)
                    if not trivial_gb:
                        for k in range(NDCH):
                            nc.vector.tensor_scalar(o[:, k], o[:, k],
                                                    g_sb[:, k:k + 1], b_sb[:, k:k + 1],
                                                    ALU.mult, ALU.add)

            # ---------- Phase 2: support-side tensors ----------
            with tc.tile_pool(name="sprep", bufs=2) as sprep, \
                 tc.tile_pool(name="svt", bufs=1) as svtp, \
                 tc.tile_pool(name="pp_s", bufs=2, space="PSUM") as pp_s, \
                 tc.tile_pool(name="pp_t", bufs=4, space="PSUM") as pp_t:
                skv = svtp.tile([128, 2, NDCH, nb_pad], bf16)
                # zero only the inter-class pad columns (their scores -> exp=1,
                # harmless because s_v pad rows are zero)
                start_item = 0
                for c in range(WAY):
                    n_c = int(counts[c])
                    rows = n_c * T
                    pad_lo = offs[c] + rows
                    pad_hi = offs[c + 1] if c + 1 < WAY else nb_pad
                    if pad_hi > pad_lo:
                        nc.gpsimd.memset(skv[:, :, :, pad_lo:pad_hi], 0.0)
                    dst = skv[:, :, :, offs[c]:offs[c] + rows].rearrange(
                        "p kv m (n a) -> p kv m n a", a=T)
                    gather(dst, start_item, n_c, sprep)
                    start_item += n_c
                # LayerNorm all support columns at once (pad columns are zero
                # and stay zero), writing into the persistent s_kT
                col_ln(skv[:, 0], nb_pad, sprep, pp_s, out=s_kT)
                # transpose V half [d, nb] -> s_v [nb, d] via PE
                ident = svtp.tile([128, 128], bf16)
                make_identity(nc, ident)
                for w in range(nwch):
                    for dd in range(NDCH):
                        ps = pp_t.tile([128, 128], bf16, tag="tps")
                        nc.tensor.transpose(ps, skv[:, 1, dd, w * 128:(w + 1) * 128], ident)
                        nc.vector.tensor_copy(s_v[:, w, dd * 128:(dd + 1) * 128], ps)

            # ---------- Phase 3: per-group query pipeline ----------
            # Column sums (S_c, B_c, C_c, A) are M=1 ones-matmuls packed 4 per
            # PSUM tile at partitions {0,32,64,96} via tile_position so the PE
            # runs them concurrently in distinct 32-column groups. The rows are
            # then DMA-packed into a [WAY, 4, C] tile (partition = class) so
            # the final combine runs on 5 lanes instead of 1.
            TS, TB, TC, TA = 0, 1, 2, 3  # term slots in the packed tile

            def packed_sum(ps_tile, slot, rhs_chunks, first, last):
                """Accumulate sum-over-partitions of each rhs chunk into
                ps_tile[32*slot] using a col-group tile_position."""
                out = ps_tile[32 * slot:32 * slot + 1]
                for i, (rhs, kc) in enumerate(rhs_chunks):
                    nc.tensor.matmul(out, ones_sb[:kc], rhs, start=(first and i == 0),
                                     stop=(last and i == len(rhs_chunks) - 1),
                                     tile_position=(0, 32 * slot),
                                     skip_group_check=True)

            with tc.tile_pool(name="grp", bufs=2) as grp, \
                 tc.tile_pool(name="rows", bufs=2) as rows_pool, \
                 tc.tile_pool(name="pp_sc", bufs=2, space="PSUM") as pp_sc, \
                 tc.tile_pool(name="pp_pr", bufs=2, space="PSUM") as pp_pr, \
                 tc.tile_pool(name="pp_row", bufs=2, space="PSUM") as pp_row:
                for g in range(N_GROUPS):
                    qkv = grp.tile([128, 2, NDCH, G_Q, T], bf16, tag="qkv")
                    items0 = N_SUPPORT + g * G_Q
                    gather(qkv, items0, G_Q, grp)
                    qk3 = qkv[:, 0].rearrange("p m q a -> p m (q a)")
                    qv3 = qkv[:, 1].rearrange("p m q a -> p m (q a)")
                    col_ln(qk3, C, grp, pp_row)

                    # scoresT + exp
                    exp_t = grp.tile([128, nwch, C], bf16, tag="exp")
                    for w in range(nwch):
                        ps = pp_sc.tile([128, C], f32, tag="scps")
                        for k in range(NDCH):
                            nc.tensor.matmul(ps, s_kT[:, k, w * 128:(w + 1) * 128],
                                             qk3[:, k], start=(k == 0), stop=(k == NDCH - 1))
                        nc.scalar.activation(exp_t[:, w], ps, AF.Exp, scale=inv_sqrt)

                    packed = rows_pool.tile([WAY, 4, C], f32, tag="packed")

                    # A = ||q_v||^2 per column
                    qsq = grp.tile([128, NDCH, C], bf16, tag="qsq", bufs=1)
                    nc.scalar.activation(qsq, qv3, AF.Square)
                    ps_a = pp_row.tile([128, C], f32, tag="sumps", name="ps_a")
                    packed_sum(ps_a, 0, [(qsq[:, k], 128) for k in range(NDCH)], True, True)
                    a_sb = rows_pool.tile([1, C], f32, tag="a_sb", bufs=1)
                    nc.vector.tensor_copy(a_sb, ps_a[0:1])
                    a5 = rows_pool.tile([WAY, C], f32, tag="a5", bufs=1)
                    nc.gpsimd.partition_broadcast(a5, a_sb)

                    def s_chunks(c):
                        rows = int(counts[c]) * T
                        wlo = offs[c] // 128
                        return [(exp_t[:min(128, rows - wi * 128), wlo + wi],
                                 min(128, rows - wi * 128))
                                for wi in range((rows + 127) // 128)]

                    def stage_rows(ps, rows):
                        """PSUM sum-tile -> SBUF (one lane-parallel copy; engines
                        cannot address partition starts other than 0/32/64/96),
                        then DMA rows {32j} into packed[class, term]."""
                        st = rows_pool.tile([128, C], f32, tag="stage", name="stage")
                        nc.scalar.activation(st, ps, AF.Copy)
                        st4 = st.rearrange("(j z) n -> j z n", z=32)
                        for j, (cc, term) in enumerate(rows):
                            nc.sync.dma_start(packed[cc:cc + 1, term],
                                              st4[j:j + 1, 0, :])

                    # S_0..S_3 packed in one PSUM tile (concurrent col-groups)
                    ps_s03 = pp_row.tile([128, C], f32, tag="sumps", name="ps_s03")
                    nchunks = max(len(s_chunks(c)) for c in range(4))
                    for i in range(nchunks):
                        for c in range(4):
                            ch = s_chunks(c)
                            if i < len(ch):
                                packed_sum(ps_s03, c, [ch[i]], i == 0, i == len(ch) - 1)
                    stage_rows(ps_s03, [(0, TS), (1, TS), (2, TS), (3, TS)])

                    # remaining sum streams: S_4, then B_c/C_c per class,
                    # packed 4 per PSUM tile
                    ps_bc = pp_row.tile([128, C], f32, tag="sumps", name="ps_bc0")
                    packed_sum(ps_bc, 0, s_chunks(4), True, True)
                    pending = [(4, TS)]

                    def bc_flush(force=False):
                        nonlocal ps_bc, pending
                        if pending and (force or len(pending) >= 3):
                            stage_rows(ps_bc, pending)
                            pending = []
                            if not force:
                                ps_bc = pp_row.tile([128, C], f32, tag="sumps",
                                                    name="ps_bc")
                    for c in range(WAY):
                        rows = int(counts[c]) * T
                        wlo = offs[c] // 128
                        nw_c = (rows + 127) // 128
                        # prototypes: P[d, col] accumulated over class rows
                        pt = grp.tile([128, NDCH, C], bf16, tag="pt", bufs=1)
                        p2t = grp.tile([128, NDCH, C], bf16, tag="p2t", bufs=1)
                        for dd in range(NDCH):
                            ps_p = pp_pr.tile([128, C], f32, tag="prps")
                            for wi in range(nw_c):
                                nc.tensor.matmul(ps_p, s_v[:, wlo + wi, dd * 128:(dd + 1) * 128],
                                                 exp_t[:, wlo + wi],
                                                 start=(wi == 0), stop=(wi == nw_c - 1))
                            if dd % 2 == 0:
                                nc.scalar.activation(pt[:, dd], ps_p, AF.Copy)
                            else:
                                nc.vector.tensor_copy(pt[:, dd], ps_p)
                        nc.scalar.activation(p2t, pt, AF.Square)
                        nc.gpsimd.tensor_mul(pt, pt, qv3)  # now <q_v, P> terms
                        # interleave B_c / C_c chunk streams for PE concurrency
                        sb, sc = len(pending), len(pending) + 1
                        for k in range(NDCH):
                            packed_sum(ps_bc, sb, [(pt[:, k], 128)], k == 0, k == NDCH - 1)
                            packed_sum(ps_bc, sc, [(p2t[:, k], 128)], k == 0, k == NDCH - 1)
                        pending += [(c, TB), (c, TC)]
                        bc_flush()
                    bc_flush(force=True)

                    # dist = A - 2 B/S + C/S^2 ; logits = -sum_a dist / T
                    sinv = rows_pool.tile([WAY, C], f32, tag="sinv", bufs=1)
                    nc.vector.reciprocal(sinv, packed[:, TS])
                    u = rows_pool.tile([WAY, C], f32, tag="u", bufs=1)
                    nc.vector.tensor_mul(u, packed[:, TC, :], sinv)
                    nc.vector.scalar_tensor_tensor(u, packed[:, TB, :], -2.0, u,
                                                   ALU.mult, ALU.add)
                    nc.vector.tensor_mul(u, u, sinv)
                    nc.vector.tensor_add(u, u, a5)
                    u4 = u.rearrange("w (q a) -> w q a", a=T)
                    red = rows_pool.tile([WAY, G_Q], f32, tag="red", bufs=1)
                    nc.vector.reduce_sum(red, u4, mybir.AxisListType.X)
                    nc.scalar.activation(logits5[:, g * G_Q:(g + 1) * G_Q], red,
                                         AF.Copy, scale=-1.0 / T)

            nc.sync.dma_start(out_d.rearrange("q c -> c q"), logits5)

    nc.compile()
    return nc


def kernel(support_set, support_labels, queries, k_w, k_b, v_w, v_b, ln_g, ln_b):
    import concourse.bass_utils as bass_utils

    support_set = np.asarray(support_set, dtype=np.float32)
    queries = np.asarray(queries, dtype=np.float32)
    labels = np.asarray(support_labels, dtype=np.int32)
    k_w = np.asarray(k_w, dtype=np.float32)
    v_w = np.asarray(v_w, dtype=np.float32)
    k_b = np.asarray(k_b, dtype=np.float32)
    v_b = np.asarray(v_b, dtype=np.float32)
    ln_g = np.asarray(ln_g, dtype=np.float32)
    ln_b = np.asarray(ln_b, dtype=np.float32)

    pe = _pos_encoding()
    s = support_set + pe[None]
    q = queries + pe[None]
    order = np.argsort(labels, kind="stable")
    counts = np.bincount(labels, minlength=WAY)
    s_sorted = s[order]
    trivial_gb = bool(np.all(ln_g == 1.0) and np.all(ln_b == 0.0))

    key = (tuple(int(x) for x in counts), trivial_gb)
    if key not in _CACHE:
        _CACHE[key] = _build_kernel(counts, trivial_gb)
    nc = _CACHE[key]

    W = np.zeros((KPAD, 6 * OUT_DIM), np.float32)
    for j in range(TSS):
        W[:IN_DIM, j * OUT_DIM:(j + 1) * OUT_DIM] = k_w[j * IN_DIM:(j + 1) * IN_DIM]
        W[:IN_DIM, (TSS + j) * OUT_DIM:(TSS + j + 1) * OUT_DIM] = v_w[j * IN_DIM:(j + 1) * IN_DIM]
        W[IN_DIM, j * OUT_DIM:(j + 1) * OUT_DIM] = k_b / TSS
        W[IN_DIM, (TSS + j) * OUT_DIM:(TSS + j + 1) * OUT_DIM] = v_b / TSS
    w_perm = np.ascontiguousarray(
        W.reshape(NKCH, 128, NMB, 128).transpose(1, 2, 0, 3)).astype(BF16)
    g_in = np.ascontiguousarray(ln_g.reshape(NDCH, 128).T).astype(BF16)
    b_in = np.ascontiguousarray(ln_b.reshape(NDCH, 128).T).astype(BF16)

    in_maps = []
    for core in range(N_CORES):
        qs = q[core * NQL:(core + 1) * NQL]
        X = np.concatenate([s_sorted.reshape(-1, IN_DIM), qs.reshape(-1, IN_DIM)], 0)
        XT = np.zeros((KPAD, NX), np.float32)
        XT[:IN_DIM] = X.T
        XT[IN_DIM] = 1.0
        x_perm = np.ascontiguousarray(
            XT.reshape(NKCH, 128, NX).transpose(1, 0, 2)).astype(BF16)
        in_maps.append({"x": x_perm, "w": w_perm, "lng": g_in, "lnb": b_in})

    global _LAST_IN_MAPS
    _LAST_IN_MAPS = in_maps
    res = bass_utils.run_bass_kernel_spmd(nc, in_maps, core_ids=list(range(N_CORES)))
    return np.concatenate([res.results[i]["out"] for i in range(N_CORES)], 0)


_LAST_IN_MAPS = None


# revision 25
# speedup vs baseline: 1.2179x; 1.2179x over previous
"""Trainium2 Bass kernel for the CNN-TRX few-shot attention head.

Sharding: data-parallel over the 200 queries (25 per NeuronCore); support set
and weights replicated per core. All matmuls in bf16 with fp32 PSUM:

  1. Frame projection in transposed layout: f_T[d, frame] for all 6 weight
     blocks (k_w/v_w x 3 tuple positions); biases folded via an augmented
     ones-row of X.
  2. Tuple gather (C(8,3)=56 frame triples) as 2-stage DVE column adds.
  3. LayerNorm of K projections column-wise: stats via ones-matmuls, Rsqrt on
     ACT, gpsimd partition-broadcast, two DVE passes.
  4. scoresT = s_k_pad^T q_k with supports sorted by class and class blocks
     padded to 128 rows; exp via ACT (no max-subtract: LN'd scores are O(1),
     Cauchy-Schwarz bounds |score| <= 34 so exp stays finite in fp32).
  5. Per-class prototypes in T-layout; distance terms ||q_v||^2, <q_v,P>,
     ||P||^2, sum(exp) via ones-matmul column reductions; final combine on
     single-partition rows; logits = -sum_a dist / 56.
"""

import math
from itertools import combinations

import ml_dtypes
import numpy as np

SEQ = 8
IN_DIM = 2048
OUT_DIM = 1152
TSS = 3
WAY = 5
N_SUPPORT = 25
N_QUERIES = 200
PE_SCALE = 0.1
LN_EPS = 1e-5
T = 56
N_CORES = 8
NQL = N_QUERIES // N_CORES      # queries per core
G_Q = 5                         # queries per inner group
N_GROUPS = NQL // G_Q
C = G_Q * T                     # score columns per group (280)
KPAD = 2176                     # 17 * 128 (2048 data + ones row + zero pad)
NKCH = KPAD // 128
NDCH = OUT_DIM // 128           # 9
NMB = 6 * OUT_DIM // 128        # 54 projection column blocks
NX = SEQ * 2 * N_SUPPORT        # 400 frame columns per core
PAIRS = [(t0, t1) for t0 in range(SEQ - 2) for t1 in range(t0 + 1, SEQ - 1)]
LN_CHUNK = 448                  # LayerNorm column chunk (PSUM free-dim <= 512)
BF16 = ml_dtypes.bfloat16

_CACHE = {}


def _pos_encoding():
    pos = np.arange(SEQ, dtype=np.float32)[:, None]
    div = np.exp(np.arange(0, IN_DIM, 2, dtype=np.float32) * -(math.log(10000.0) / IN_DIM))
    pe = np.zeros((SEQ, IN_DIM), dtype=np.float32)
    pe[:, 0::2] = np.sin(pos * div) * PE_SCALE
    pe[:, 1::2] = np.cos(pos * div) * PE_SCALE
    return pe


def _class_layout(counts):
    offs, off = [], 0
    for c in range(WAY):
        offs.append(off)
        off += ((counts[c] * T + 127) // 128) * 128
    return offs, off


def _build_kernel(counts, trivial_gb):
    import concourse.mybir as mybir
    import concourse.tile as tile
    from concourse import bacc
    from concourse.masks import make_identity

    f32 = mybir.dt.float32
    bf16 = mybir.dt.bfloat16
    AF = mybir.ActivationFunctionType
    ALU = mybir.AluOpType
    offs, nb_pad = _class_layout(counts)
    nwch = nb_pad // 128
    inv_sqrt = 1.0 / math.sqrt(OUT_DIM)

    nc = bacc.Bacc("TRN2", target_bir_lowering=False, debug=False,
                   enable_asserts=False, num_devices=N_CORES)

    x_d = nc.dram_tensor("x", [128, NKCH, NX], bf16, kind="ExternalInput").ap()
    w_d = nc.dram_tensor("w", [128, NMB, NKCH, 128], bf16, kind="ExternalInput").ap()
    g_d = nc.dram_tensor("lng", [128, NDCH], bf16, kind="ExternalInput").ap()
    b_d = nc.dram_tensor("lnb", [128, NDCH], bf16, kind="ExternalInput").ap()
    out_d = nc.dram_tensor("out", [NQL, WAY], f32, kind="ExternalOutput").ap()

    with tile.TileContext(nc) as tc:
        with tc.tile_pool(name="big", bufs=1) as big, \
             tc.tile_pool(name="small", bufs=1) as small:
            f_T = big.tile([128, 6 * NDCH, NX], bf16)       # frame projections, T-layout
            s_kT = big.tile([128, NDCH, nb_pad], bf16)      # LN'd support K, padded cols
            s_v = big.tile([128, nwch, OUT_DIM], bf16)      # support V, row-natural padded
            ones_sb = small.tile([128, 1], bf16)
            nc.vector.memset(ones_sb, 1.0)
            eps_sb = small.tile([1, 1], f32)
            nc.vector.memset(eps_sb, LN_EPS)
            g_sb = small.tile([128, NDCH], bf16)
            b_sb = small.tile([128, NDCH], bf16)
            nc.sync.dma_start(g_sb, g_d)
            nc.sync.dma_start(b_sb, b_d)
            logits5 = small.tile([WAY, NQL], f32)

            # ---------- Phase 1: frame projections ----------
            with tc.tile_pool(name="xt_pool", bufs=1) as xt_pool, \
                 tc.tile_pool(name="xw", bufs=3) as xw, \
                 tc.tile_pool(name="pp_proj", bufs=4, space="PSUM") as pp_proj:
                xt = xt_pool.tile([128, NKCH, NX], bf16)
                nc.sync.dma_start(xt, x_d)
                for m in range(NMB):
                    wm = xw.tile([128, NKCH, 128], bf16, tag="wslab")
                    nc.sync.dma_start(wm, w_d[:, m])
                    ps = pp_proj.tile([128, NX], f32, tag="projps")
                    for k in range(NKCH):
                        nc.tensor.matmul(ps, wm[:, k], xt[:, k],
                                         start=(k == 0), stop=(k == NKCH - 1))
                    nc.scalar.activation(f_T[:, m], ps, AF.Copy)

            # f_T blocks: m = kv*27 + j*NDCH + dd
            f6 = f_T.rearrange("p (kv j d) (i s) -> p kv j d i s", kv=2, j=TSS, s=SEQ)

            def gather(dst5, items0, n_items, pool):
                """dst5 [128, 2, NDCH, n_items, T] = tuple-gathered frame
                projections for the K and V paths. DVE ISA allows at most 3
                free AP dims, so K/V are separate ops."""
                isl = slice(items0, items0 + n_items)
                p2 = pool.tile([128, 2, NDCH, n_items, len(PAIRS)], bf16, tag="pairs")
                for kv in range(2):
                    pi = 0
                    for t0 in range(SEQ - 2):
                        run = SEQ - 2 - t0
                        a = f6[:, kv, 0, :, isl, t0:t0 + 1]
                        b = f6[:, kv, 1, :, isl, t0 + 1:t0 + 1 + run]
                        nc.vector.tensor_add(p2[:, kv, :, :, pi:pi + run],
                                             a.to_broadcast(b.shape), b)
                        pi += run
                    ai = 0
                    for pi, (t0, t1) in enumerate(PAIRS):
                        run = SEQ - 1 - t1
                        a = p2[:, kv, :, :, pi:pi + 1]
                        b = f6[:, kv, 2, :, isl, t1 + 1:t1 + 1 + run]
                        nc.vector.tensor_add(dst5[:, kv, :, :, ai:ai + run],
                                             a.to_broadcast(b.shape), b)
                        ai += run

            def col_ln(raw, cols, pool, psum_pool, out=None):
                """Column-wise LayerNorm of raw [128, NDCH, cols] (T-layout);
                in place unless `out` is given."""
                if out is None:
                    out = raw
                for c0 in range(0, cols, LN_CHUNK):
                    cw = min(LN_CHUNK, cols - c0)
                    r = raw[:, :, c0:c0 + cw]
                    o = out[:, :, c0:c0 + cw]
                    sq = pool.tile([128, NDCH, cw], bf16, tag="lnsq", name="lnsq", bufs=1)
                    nc.scalar.activation(sq, r, AF.Square)
                    ps_s = psum_pool.tile([1, cw], f32, tag="lnps", name="lnps")
                    ps_q = psum_pool.tile([1, cw], f32, tag="lnps", name="lnps")
                    for k in range(NDCH):
                        nc.tensor.matmul(ps_s, ones_sb, r[:, k],
                                         start=(k == 0), stop=(k == NDCH - 1))
                    for k in range(NDCH):
                        nc.tensor.matmul(ps_q, ones_sb, sq[:, k],
                                         start=(k == 0), stop=(k == NDCH - 1))
                    m_r = pool.tile([1, cw], f32, tag="lnm", name="lnm")
                    v_r = pool.tile([1, cw], f32, tag="lnv", name="lnv")
                    mm = pool.tile([1, cw], f32, tag="lnmm", name="lnmm")
                    nc.scalar.activation(m_r, ps_s, AF.Copy, scale=1.0 / OUT_DIM)
                    nc.scalar.activation(v_r, ps_q, AF.Copy, scale=1.0 / OUT_DIM)
                    nc.vector.tensor_mul(mm, m_r, m_r)
                    nc.vector.tensor_sub(v_r, v_r, mm)
                    nc.scalar.activation(v_r, v_r, AF.Sqrt, bias=eps_sb)
                    nc.vector.reciprocal(v_r, v_r)
                    m_b = pool.tile([128, cw], f32, tag="lnmb", name="lnmb", bufs=1)
                    a_b = pool.tile([128, cw], f32, tag="lnab", name="lnab", bufs=1)
                    nc.gpsimd.partition_broadcast(m_b, m_r)
                    nc.gpsimd.partition_broadcast(a_b, v_r)
                    mb3 = m_b[:, None, :].to_broadcast([128, NDCH, cw])
                    ab3 = a_b[:, None, :].to_broadcast([128, NDCH, cw])
                    nc.vector.tensor_sub(r, r, mb3)
                    nc.vector.tensor_mul(o, r, ab3)
                    if not trivial_gb:
                        for k in range(NDCH):
                            nc.vector.tensor_scalar(o[:, k], o[:, k],
                                                    g_sb[:, k:k + 1], b_sb[:, k:k + 1],
                                                    ALU.mult, ALU.add)

            # ---------- Phase 2: support-side tensors ----------
            with tc.tile_pool(name="sprep", bufs=2) as sprep, \
                 tc.tile_pool(name="svt", bufs=1) as svtp, \
                 tc.tile_pool(name="pp_s", bufs=2, space="PSUM") as pp_s, \
                 tc.tile_pool(name="pp_t", bufs=4, space="PSUM") as pp_t:
                skv = svtp.tile([128, 2, NDCH, nb_pad], bf16)
                # zero only the inter-class pad columns (their scores -> exp=1,
                # harmless because s_v pad rows are zero)
                start_item = 0
                for c in range(WAY):
                    n_c = int(counts[c])
                    rows = n_c * T
                    pad_lo = offs[c] + rows
                    pad_hi = offs[c + 1] if c + 1 < WAY else nb_pad
                    if pad_hi > pad_lo:
                        nc.gpsimd.memset(skv[:, :, :, pad_lo:pad_hi], 0.0)
                    dst = skv[:, :, :, offs[c]:offs[c] + rows].rearrange(
                        "p kv m (n a) -> p kv m n a", a=T)
                    gather(dst, start_item, n_c, sprep)
                    start_item += n_c
                # LayerNorm all support columns at once (pad columns are zero
                # and stay zero), writing into the persistent s_kT
                col_ln(skv[:, 0], nb_pad, sprep, pp_s, out=s_kT)
                # transpose V half [d, nb] -> s_v [nb, d] via PE
                ident = svtp.tile([128, 128], bf16)
                make_identity(nc, ident)
                for w in range(nwch):
                    for dd in range(NDCH):
                        ps = pp_t.tile([128, 128], bf16, tag="tps")
                        nc.tensor.transpose(ps, skv[:, 1, dd, w * 128:(w + 1) * 128], ident)
                        nc.vector.tensor_copy(s_v[:, w, dd * 128:(dd + 1) * 128], ps)

            # ---------- Phase 3: per-group query pipeline ----------
            # Column sums (S_c, B_c, C_c, A) are M=1 ones-matmuls packed 4 per
            # PSUM tile at partitions {0,32,64,96} via tile_position so the PE
            # runs them concurrently in distinct 32-column groups. The rows are
            # then DMA-packed into a [WAY, 4, C] tile (partition = class) so
            # the final combine runs on 5 lanes instead of 1.
            TS, TB, TC, TA = 0, 1, 2, 3  # term slots in the packed tile

            def packed_sum(ps_tile, slot, rhs_chunks, first, last):
                """Accumulate sum-over-partitions of each rhs chunk into
                ps_tile[32*slot] using a col-group tile_position."""
                out = ps_tile[32 * slot:32 * slot + 1]
                for i, (rhs, kc) in enumerate(rhs_chunks):
                    nc.tensor.matmul(out, ones_sb[:kc], rhs, start=(first and i == 0),
                                     stop=(last and i == len(rhs_chunks) - 1),
                                     tile_position=(0, 32 * slot),
                                     skip_group_check=True)

            with tc.tile_pool(name="grp", bufs=2) as grp, \
                 tc.tile_pool(name="rows", bufs=2) as rows_pool, \
                 tc.tile_pool(name="pp_sc", bufs=2, space="PSUM") as pp_sc, \
                 tc.tile_pool(name="pp_pr", bufs=2, space="PSUM") as pp_pr, \
                 tc.tile_pool(name="pp_row", bufs=2, space="PSUM") as pp_row:
                for g in range(N_GROUPS):
                    qkv = grp.tile([128, 2, NDCH, G_Q, T], bf16, tag="qkv")
                    items0 = N_SUPPORT + g * G_Q
                    gather(qkv, items0, G_Q, grp)
                    qk3 = qkv[:, 0].rearrange("p m q a -> p m (q a)")
                    qv3 = qkv[:, 1].rearrange("p m q a -> p m (q a)")
                    col_ln(qk3, C, grp, pp_row)

                    # scoresT + exp
                    exp_t = grp.tile([128, nwch, C], bf16, tag="exp")
                    for w in range(nwch):
                        ps = pp_sc.tile([128, C], f32, tag="scps")
                        for k in range(NDCH):
                            nc.tensor.matmul(ps, s_kT[:, k, w * 128:(w + 1) * 128],
                                             qk3[:, k], start=(k == 0), stop=(k == NDCH - 1))
                        nc.scalar.activation(exp_t[:, w], ps, AF.Exp, scale=inv_sqrt)

                    packed = rows_pool.tile([WAY, 4, C], f32, tag="packed")

                    # A = ||q_v||^2 per column
                    qsq = grp.tile([128, NDCH, C], bf16, tag="qsq", bufs=1)
                    nc.scalar.activation(qsq, qv3, AF.Square)
                    ps_a = pp_row.tile([128, C], f32, tag="sumps", name="ps_a")
                    packed_sum(ps_a, 0, [(qsq[:, k], 128) for k in range(NDCH)], True, True)
                    a_sb = rows_pool.tile([1, C], f32, tag="a_sb", bufs=1)
                    nc.vector.tensor_copy(a_sb, ps_a[0:1])
                    a5 = rows_pool.tile([WAY, C], f32, tag="a5", bufs=1)
                    nc.gpsimd.partition_broadcast(a5, a_sb)

                    def s_chunks(c):
                        rows = int(counts[c]) * T
                        wlo = offs[c] // 128
                        return [(exp_t[:min(128, rows - wi * 128), wlo + wi],
                                 min(128, rows - wi * 128))
                                for wi in range((rows + 127) // 128)]

                    def stage_rows(ps, rows):
                        """PSUM sum-tile -> SBUF (one lane-parallel copy; engines
                        cannot address partition starts other than 0/32/64/96),
                        then DMA rows {32j} into packed[class, term]."""
                        st = rows_pool.tile([128, C], f32, tag="stage", name="stage")
                        nc.scalar.activation(st, ps, AF.Copy)
                        st4 = st.rearrange("(j z) n -> j z n", z=32)
                        for j, (cc, term) in enumerate(rows):
                            nc.sync.dma_start(packed[cc:cc + 1, term],
                                              st4[j:j + 1, 0, :])

                    # S_0..S_3 packed in one PSUM tile (concurrent col-groups)
                    ps_s03 = pp_row.tile([128, C], f32, tag="sumps", name="ps_s03")
                    nchunks = max(len(s_chunks(c)) for c in range(4))
                    for i in range(nchunks):
                        for c in range(4):
                            ch = s_chunks(c)
                            if i < len(ch):
                                packed_sum(ps_s03, c, [ch[i]], i == 0, i == len(ch) - 1)
                    stage_rows(ps_s03, [(0, TS), (1, TS), (2, TS), (3, TS)])

                    # remaining sum streams: S_4, then B_c/C_c per class,
                    # packed 4 per PSUM tile
                    ps_bc = pp_row.tile([128, C], f32, tag="sumps", name="ps_bc0")
                    packed_sum(ps_bc, 0, s_chunks(4), True, True)
                    pending = [(4, TS)]

                    def bc_flush(force=False):
                        nonlocal ps_bc, pending
                        if pending and (force or len(pending) >= 3):
                            stage_rows(ps_bc, pending)
                            pending = []
                            if not force:
                                ps_bc = pp_row.tile([128, C], f32, tag="sumps",
                                                    name="ps_bc")
                    for c in range(WAY):
                        rows = int(counts[c]) * T
                        wlo = offs[c] // 128
                        nw_c = (rows + 127) // 128
                        # prototypes: P[d, col] accumulated over class rows
                        pt = grp.tile([128, NDCH, C], bf16, tag="pt", bufs=1)
                        p2t = grp.tile([128, NDCH, C], bf16, tag="p2t", bufs=1)
                        for dd in range(NDCH):
                            ps_p = pp_pr.tile([128, C], f32, tag="prps")
                            for wi in range(nw_c):
                                nc.tensor.matmul(ps_p, s_v[:, wlo + wi, dd * 128:(dd + 1) * 128],
                                                 exp_t[:, wlo + wi],
                                                 start=(wi == 0), stop=(wi == nw_c - 1))
                            if dd % 2 == 0:
                                nc.scalar.activation(pt[:, dd], ps_p, AF.Copy)
                            else:
                                nc.vector.tensor_copy(pt[:, dd], ps_p)
                        nc.scalar.activation(p2t, pt, AF.Square)
                        nc.vector.tensor_mul(pt, pt, qv3)  # now <q_v, P> terms
                        # interleave B_c / C_c chunk streams for PE concurrency
                        sb, sc = len(pending), len(pending) + 1
                        for k in range(NDCH):
                            packed_sum(ps_bc, sb, [(pt[:, k], 128)], k == 0, k == NDCH - 1)
                            packed_sum(ps_bc, sc, [(p2t[:, k], 128)], k == 0, k == NDCH - 1)
                        pending += [(c, TB), (c, TC)]
                        bc_flush()
                    bc_flush(force=True)

                    # dist = A - 2 B/S + C/S^2 ; logits = -sum_a dist / T
                    sinv = rows_pool.tile([WAY, C], f32, tag="sinv", bufs=1)
                    nc.vector.reciprocal(sinv, packed[:, TS])
                    u = rows_pool.tile([WAY, C], f32, tag="u", bufs=1)
                    nc.vector.tensor_mul(u, packed[:, TC, :], sinv)
                    nc.vector.scalar_tensor_tensor(u, packed[:, TB, :], -2.0, u,
                                                   ALU.mult, ALU.add)
                    nc.vector.tensor_mul(u, u, sinv)
                    nc.vector.tensor_add(u, u, a5)
                    u4 = u.rearrange("w (q a) -> w q a", a=T)
                    red = rows_pool.tile([WAY, G_Q], f32, tag="red", bufs=1)
                    nc.vector.reduce_sum(red, u4, mybir.AxisListType.X)
                    nc.scalar.activation(logits5[:, g * G_Q:(g + 1) * G_Q], red,
                                         AF.Copy, scale=-1.0 / T)

            nc.sync.dma_start(out_d.rearrange("q c -> c q"), logits5)

    nc.compile()
    return nc


def kernel(support_set, support_labels, queries, k_w, k_b, v_w, v_b, ln_g, ln_b):
    import concourse.bass_utils as bass_utils

    support_set = np.asarray(support_set, dtype=np.float32)
    queries = np.asarray(queries, dtype=np.float32)
    labels = np.asarray(support_labels, dtype=np.int32)
    k_w = np.asarray(k_w, dtype=np.float32)
    v_w = np.asarray(v_w, dtype=np.float32)
    k_b = np.asarray(k_b, dtype=np.float32)
    v_b = np.asarray(v_b, dtype=np.float32)
    ln_g = np.asarray(ln_g, dtype=np.float32)
    ln_b = np.asarray(ln_b, dtype=np.float32)

    pe = _pos_encoding()
    s = support_set + pe[None]
    q = queries + pe[None]
    order = np.argsort(labels, kind="stable")
    counts = np.bincount(labels, minlength=WAY)
    s_sorted = s[order]
    trivial_gb = bool(np.all(ln_g == 1.0) and np.all(ln_b == 0.0))

    key = (tuple(int(x) for x in counts), trivial_gb)
    if key not in _CACHE:
        _CACHE[key] = _build_kernel(counts, trivial_gb)
    nc = _CACHE[key]

    W = np.zeros((KPAD, 6 * OUT_DIM), np.float32)
    for j in range(TSS):
        W[:IN_DIM, j * OUT_DIM:(j + 1) * OUT_DIM] = k_w[j * IN_DIM:(j + 1) * IN_DIM]
        W[:IN_DIM, (TSS + j) * OUT_DIM:(TSS + j + 1) * OUT_DIM] = v_w[j * IN_DIM:(j + 1) * IN_DIM]
        W[IN_DIM, j * OUT_DIM:(j + 1) * OUT_DIM] = k_b / TSS
        W[IN_DIM, (TSS + j) * OUT_DIM:(TSS + j + 1) * OUT_DIM] = v_b / TSS
    w_perm = np.ascontiguousarray(
        W.reshape(NKCH, 128, NMB, 128).transpose(1, 2, 0, 3)).astype(BF16)
    g_in = np.ascontiguousarray(ln_g.reshape(NDCH, 128).T).astype(BF16)
    b_in = np.ascontiguousarray(ln_b.reshape(NDCH, 128).T).astype(BF16)

    in_maps = []
    for core in range(N_CORES):
        qs = q[core * NQL:(core + 1) * NQL]
        X = np.concatenate([s_sorted.reshape(-1, IN_DIM), qs.reshape(-1, IN_DIM)], 0)
        XT = np.zeros((KPAD, NX), np.float32)
        XT[:IN_DIM] = X.T
        XT[IN_DIM] = 1.0
        x_perm = np.ascontiguousarray(
            XT.reshape(NKCH, 128, NX).transpose(1, 0, 2)).astype(BF16)
        in_maps.append({"x": x_perm, "w": w_perm, "lng": g_in, "lnb": b_in})

    global _LAST_IN_MAPS
    _LAST_IN_MAPS = in_maps
    res = bass_utils.run_bass_kernel_spmd(nc, in_maps, core_ids=list(range(N_CORES)))
    return np.concatenate([res.results[i]["out"] for i in range(N_CORES)], 0)


_LAST_IN_MAPS = None


# revision 29
# speedup vs baseline: 1.2387x; 1.0171x over previous
"""Trainium2 Bass kernel for the CNN-TRX few-shot attention head.

Sharding: data-parallel over the 200 queries (25 per NeuronCore); support set
and weights replicated per core. All matmuls in bf16 with fp32 PSUM:

  1. Frame projection in transposed layout: f_T[d, frame] for all 6 weight
     blocks (k_w/v_w x 3 tuple positions); biases folded via an augmented
     ones-row of X.
  2. Tuple gather (C(8,3)=56 frame triples) as 2-stage DVE column adds.
  3. LayerNorm of K projections column-wise: stats via ones-matmuls, Rsqrt on
     ACT, gpsimd partition-broadcast, two DVE passes.
  4. scoresT = s_k_pad^T q_k with supports sorted by class and class blocks
     padded to 128 rows; exp via ACT (no max-subtract: LN'd scores are O(1),
     Cauchy-Schwarz bounds |score| <= 34 so exp stays finite in fp32).
  5. Per-class prototypes in T-layout; distance terms ||q_v||^2, <q_v,P>,
     ||P||^2, sum(exp) via ones-matmul column reductions; final combine on
     single-partition rows; logits = -sum_a dist / 56.
"""

import math
from itertools import combinations

import ml_dtypes
import numpy as np

SEQ = 8
IN_DIM = 2048
OUT_DIM = 1152
TSS = 3
WAY = 5
N_SUPPORT = 25
N_QUERIES = 200
PE_SCALE = 0.1
LN_EPS = 1e-5
T = 56
N_CORES = 8
NQL = N_QUERIES // N_CORES      # queries per core
G_Q = 5                         # queries per inner group
N_GROUPS = NQL // G_Q
C = G_Q * T                     # score columns per group (280)
KPAD = 2176                     # 17 * 128 (2048 data + ones row + zero pad)
NKCH = KPAD // 128
NDCH = OUT_DIM // 128           # 9
NMB = 6 * OUT_DIM // 128        # 54 projection column blocks
NX = SEQ * 2 * N_SUPPORT        # 400 frame columns per core
PAIRS = [(t0, t1) for t0 in range(SEQ - 2) for t1 in range(t0 + 1, SEQ - 1)]
LN_CHUNK = 448                  # LayerNorm column chunk (PSUM free-dim <= 512)
BF16 = ml_dtypes.bfloat16

_CACHE = {}


def _pos_encoding():
    pos = np.arange(SEQ, dtype=np.float32)[:, None]
    div = np.exp(np.arange(0, IN_DIM, 2, dtype=np.float32) * -(math.log(10000.0) / IN_DIM))
    pe = np.zeros((SEQ, IN_DIM), dtype=np.float32)
    pe[:, 0::2] = np.sin(pos * div) * PE_SCALE
    pe[:, 1::2] = np.cos(pos * div) * PE_SCALE
    return pe


def _class_layout(counts):
    offs, off = [], 0
    for c in range(WAY):
        offs.append(off)
        off += ((counts[c] * T + 127) // 128) * 128
    return offs, off


def _build_kernel(counts, trivial_gb):
    import concourse.mybir as mybir
    import concourse.tile as tile
    from concourse import bacc
    from concourse.masks import make_identity

    f32 = mybir.dt.float32
    bf16 = mybir.dt.bfloat16
    AF = mybir.ActivationFunctionType
    ALU = mybir.AluOpType
    offs, nb_pad = _class_layout(counts)
    nwch = nb_pad // 128
    inv_sqrt = 1.0 / math.sqrt(OUT_DIM)

    nc = bacc.Bacc("TRN2", target_bir_lowering=False, debug=False,
                   enable_asserts=False, num_devices=N_CORES)

    x_d = nc.dram_tensor("x", [128, NKCH, NX], bf16, kind="ExternalInput").ap()
    w_d = nc.dram_tensor("w", [128, NMB, NKCH, 128], bf16, kind="ExternalInput").ap()
    g_d = nc.dram_tensor("lng", [128, NDCH], bf16, kind="ExternalInput").ap()
    b_d = nc.dram_tensor("lnb", [128, NDCH], bf16, kind="ExternalInput").ap()
    out_d = nc.dram_tensor("out", [NQL, WAY], f32, kind="ExternalOutput").ap()

    with tile.TileContext(nc) as tc:
        with tc.tile_pool(name="big", bufs=1) as big, \
             tc.tile_pool(name="small", bufs=1) as small:
            # frame projections, T-layout; one tile per weight block so
            # consumers only wait for the blocks they read (Tile tracks
            # dependencies per tile, not per region)
            f_b = [big.tile([128, NDCH, NX], bf16, name=f"f_b{j}") for j in range(6)]
            s_kT = big.tile([128, NDCH, nb_pad], bf16)      # LN'd support K, padded cols
            s_v = big.tile([128, nwch, OUT_DIM], bf16)      # support V, row-natural padded
            ones_sb = small.tile([128, 1], bf16)
            nc.vector.memset(ones_sb, 1.0)
            eps_sb = small.tile([1, 1], f32)
            nc.vector.memset(eps_sb, LN_EPS)
            g_sb = small.tile([128, NDCH], bf16)
            b_sb = small.tile([128, NDCH], bf16)
            nc.sync.dma_start(g_sb, g_d)
            nc.sync.dma_start(b_sb, b_d)
            logits5 = small.tile([WAY, NQL], f32)

            # ---------- Phase 1: frame projections ----------
            with tc.tile_pool(name="xt_pool", bufs=1) as xt_pool, \
                 tc.tile_pool(name="xw", bufs=3) as xw, \
                 tc.tile_pool(name="pp_proj", bufs=4, space="PSUM") as pp_proj:
                xt = xt_pool.tile([128, NKCH, NX], bf16)
                nc.sync.dma_start(xt, x_d)
                for m in range(NMB):
                    wm = xw.tile([128, NKCH, 128], bf16, tag="wslab")
                    nc.sync.dma_start(wm, w_d[:, m])
                    ps = pp_proj.tile([128, NX], f32, tag="projps")
                    for k in range(NKCH):
                        nc.tensor.matmul(ps, wm[:, k], xt[:, k],
                                         start=(k == 0), stop=(k == NKCH - 1))
                    nc.scalar.activation(f_b[m // NDCH][:, m % NDCH], ps, AF.Copy)

            f_i = [fb.rearrange("p d (i s) -> p d i s", s=SEQ) for fb in f_b]

            def gather_one(dst4, kv, items0, n_items, pool):
                """dst4 [128, NDCH, n_items, T] = tuple-gathered frame
                projections for one path (kv=0: K blocks 0-2, kv=1: V 3-5)."""
                isl = slice(items0, items0 + n_items)
                b0, b1, b2 = (f_i[3 * kv + j] for j in range(TSS))
                p2 = pool.tile([128, NDCH, n_items, len(PAIRS)], bf16,
                               tag=f"pairs{kv}", name="p2")
                pi = 0
                for t0 in range(SEQ - 2):
                    run = SEQ - 2 - t0
                    a = b0[:, :, isl, t0:t0 + 1]
                    b = b1[:, :, isl, t0 + 1:t0 + 1 + run]
                    nc.vector.tensor_add(p2[:, :, :, pi:pi + run],
                                         a.to_broadcast(b.shape), b)
                    pi += run
                ai = 0
                for pi, (t0, t1) in enumerate(PAIRS):
                    run = SEQ - 1 - t1
                    a = p2[:, :, :, pi:pi + 1]
                    b = b2[:, :, isl, t1 + 1:t1 + 1 + run]
                    nc.vector.tensor_add(dst4[:, :, :, ai:ai + run],
                                         a.to_broadcast(b.shape), b)
                    ai += run

            def col_ln(raw, cols, pool, psum_pool, out=None):
                """Column-wise LayerNorm of raw [128, NDCH, cols] (T-layout);
                in place unless `out` is given."""
                if out is None:
                    out = raw
                for c0 in range(0, cols, LN_CHUNK):
                    cw = min(LN_CHUNK, cols - c0)
                    r = raw[:, :, c0:c0 + cw]
                    o = out[:, :, c0:c0 + cw]
                    sq = pool.tile([128, NDCH, cw], bf16, tag="lnsq", name="lnsq", bufs=1)
                    nc.scalar.activation(sq, r, AF.Square)
                    ps_s = psum_pool.tile([1, cw], f32, tag="lnps", name="lnps")
                    ps_q = psum_pool.tile([1, cw], f32, tag="lnps", name="lnps")
                    for k in range(NDCH):
                        nc.tensor.matmul(ps_s, ones_sb, r[:, k],
                                         start=(k == 0), stop=(k == NDCH - 1))
                    for k in range(NDCH):
                        nc.tensor.matmul(ps_q, ones_sb, sq[:, k],
                                         start=(k == 0), stop=(k == NDCH - 1))
                    m_r = pool.tile([1, cw], f32, tag="lnm", name="lnm")
                    v_r = pool.tile([1, cw], f32, tag="lnv", name="lnv")
                    mm = pool.tile([1, cw], f32, tag="lnmm", name="lnmm")
                    nc.scalar.activation(m_r, ps_s, AF.Copy, scale=1.0 / OUT_DIM)
                    nc.scalar.activation(v_r, ps_q, AF.Copy, scale=1.0 / OUT_DIM)
                    nc.vector.tensor_mul(mm, m_r, m_r)
                    nc.vector.tensor_sub(v_r, v_r, mm)
                    nc.scalar.activation(v_r, v_r, AF.Sqrt, bias=eps_sb)
                    nc.vector.reciprocal(v_r, v_r)
                    m_b = pool.tile([128, cw], f32, tag="lnmb", name="lnmb", bufs=1)
                    a_b = pool.tile([128, cw], f32, tag="lnab", name="lnab", bufs=1)
                    nc.gpsimd.partition_broadcast(m_b, m_r)
                    nc.gpsimd.partition_broadcast(a_b, v_r)
                    mb3 = m_b[:, None, :].to_broadcast([128, NDCH, cw])
                    ab3 = a_b[:, None, :].to_broadcast([128, NDCH, cw])
                    nc.vector.tensor_sub(r, r, mb3)
                    nc.vector.tensor_mul(o, r, ab3)
                    if not trivial_gb:
                        for k in range(NDCH):
                            nc.vector.tensor_scalar(o[:, k], o[:, k],
                                                    g_sb[:, k:k + 1], b_sb[:, k:k + 1],
                                                    ALU.mult, ALU.add)

            # ---------- Phase 2: support-side tensors ----------
            with tc.tile_pool(name="sprep", bufs=2) as sprep, \
                 tc.tile_pool(name="svt", bufs=1) as svtp, \
                 tc.tile_pool(name="pp_s", bufs=2, space="PSUM") as pp_s, \
                 tc.tile_pool(name="pp_t", bufs=4, space="PSUM") as pp_t:
                s_kraw = svtp.tile([128, NDCH, nb_pad], bf16)
                s_vT = svtp.tile([128, NDCH, nb_pad], bf16)
                # zero only the inter-class pad columns (their scores -> exp=1,
                # harmless because s_v pad rows are zero)
                start_item = 0
                for c in range(WAY):
                    n_c = int(counts[c])
                    rows = n_c * T
                    pad_lo = offs[c] + rows
                    pad_hi = offs[c + 1] if c + 1 < WAY else nb_pad
                    if pad_hi > pad_lo:
                        nc.gpsimd.memset(s_kraw[:, :, pad_lo:pad_hi], 0.0)
                        nc.gpsimd.memset(s_vT[:, :, pad_lo:pad_hi], 0.0)
                    for kv, dstt in ((0, s_kraw), (1, s_vT)):
                        dst = dstt[:, :, offs[c]:offs[c] + rows].rearrange(
                            "p m (n a) -> p m n a", a=T)
                        gather_one(dst, kv, start_item, n_c, sprep)
                    start_item += n_c
                # LayerNorm all support columns at once (pad columns are zero
                # and stay zero), writing into the persistent s_kT
                col_ln(s_kraw, nb_pad, sprep, pp_s, out=s_kT)
                # transpose s_vT [d, nb] -> s_v [nb, d] via PE
                ident = svtp.tile([128, 128], bf16)
                make_identity(nc, ident)
                for w in range(nwch):
                    for dd in range(NDCH):
                        ps = pp_t.tile([128, 128], bf16, tag="tps")
                        nc.tensor.transpose(ps, s_vT[:, dd, w * 128:(w + 1) * 128], ident)
                        if dd % 2 == 0:
                            nc.vector.tensor_copy(s_v[:, w, dd * 128:(dd + 1) * 128], ps)
                        else:
                            nc.scalar.activation(s_v[:, w, dd * 128:(dd + 1) * 128],
                                                 ps, AF.Copy)

            # ---------- Phase 3: per-group query pipeline ----------
            # Column sums (S_c, B_c, C_c, A) are M=1 ones-matmuls packed 4 per
            # PSUM tile at partitions {0,32,64,96} via tile_position so the PE
            # runs them concurrently in distinct 32-column groups. The rows are
            # then DMA-packed into a [WAY, 4, C] tile (partition = class) so
            # the final combine runs on 5 lanes instead of 1.
            TS, TB, TC, TA = 0, 1, 2, 3  # term slots in the packed tile

            def packed_sum(ps_tile, slot, rhs_chunks, first, last):
                """Accumulate sum-over-partitions of each rhs chunk into
                ps_tile[32*slot] using a col-group tile_position."""
                out = ps_tile[32 * slot:32 * slot + 1]
                for i, (rhs, kc) in enumerate(rhs_chunks):
                    nc.tensor.matmul(out, ones_sb[:kc], rhs, start=(first and i == 0),
                                     stop=(last and i == len(rhs_chunks) - 1),
                                     tile_position=(0, 32 * slot),
                                     skip_group_check=True)

            with tc.tile_pool(name="grp", bufs=2) as grp, \
                 tc.tile_pool(name="rows", bufs=2) as rows_pool, \
                 tc.tile_pool(name="pp_sc", bufs=2, space="PSUM") as pp_sc, \
                 tc.tile_pool(name="pp_pr", bufs=2, space="PSUM") as pp_pr, \
                 tc.tile_pool(name="pp_row", bufs=2, space="PSUM") as pp_row:
                for g in range(N_GROUPS):
                    q_kT = grp.tile([128, NDCH, G_Q, T], bf16, tag="qk")
                    q_vT = grp.tile([128, NDCH, G_Q, T], bf16, tag="qv")
                    items0 = N_SUPPORT + g * G_Q
                    gather_one(q_kT, 0, items0, G_Q, grp)
                    gather_one(q_vT, 1, items0, G_Q, grp)
                    qk3 = q_kT.rearrange("p m q a -> p m (q a)")
                    qv3 = q_vT.rearrange("p m q a -> p m (q a)")
                    col_ln(qk3, C, grp, pp_row)

                    # scoresT + exp
                    exp_t = grp.tile([128, nwch, C], bf16, tag="exp")
                    for w in range(nwch):
                        ps = pp_sc.tile([128, C], f32, tag="scps")
                        for k in range(NDCH):
                            nc.tensor.matmul(ps, s_kT[:, k, w * 128:(w + 1) * 128],
                                             qk3[:, k], start=(k == 0), stop=(k == NDCH - 1))
                        nc.scalar.activation(exp_t[:, w], ps, AF.Exp, scale=inv_sqrt)

                    packed = rows_pool.tile([WAY, 4, C], f32, tag="packed")

                    # A = ||q_v||^2 per column
                    qsq = grp.tile([128, NDCH, C], bf16, tag="qsq", bufs=1)
                    nc.scalar.activation(qsq, qv3, AF.Square)
                    ps_a = pp_row.tile([128, C], f32, tag="sumps", name="ps_a")
                    packed_sum(ps_a, 0, [(qsq[:, k], 128) for k in range(NDCH)], True, True)
                    a_sb = rows_pool.tile([1, C], f32, tag="a_sb", bufs=1)
                    nc.vector.tensor_copy(a_sb, ps_a[0:1])
                    a5 = rows_pool.tile([WAY, C], f32, tag="a5", bufs=1)
                    nc.gpsimd.partition_broadcast(a5, a_sb)

                    def s_chunks(c):
                        rows = int(counts[c]) * T
                        wlo = offs[c] // 128
                        return [(exp_t[:min(128, rows - wi * 128), wlo + wi],
                                 min(128, rows - wi * 128))
                                for wi in range((rows + 127) // 128)]

                    def stage_rows(ps, rows):
                        """PSUM sum-tile -> SBUF (one lane-parallel copy; engines
                        cannot address partition starts other than 0/32/64/96),
                        then DMA rows {32j} into packed[class, term]."""
                        st = rows_pool.tile([128, C], f32, tag="stage", name="stage")
                        nc.scalar.activation(st, ps, AF.Copy)
                        st4 = st.rearrange("(j z) n -> j z n", z=32)
                        for j, (cc, term) in enumerate(rows):
                            nc.sync.dma_start(packed[cc:cc + 1, term],
                                              st4[j:j + 1, 0, :])

                    # S_0..S_3 packed in one PSUM tile (concurrent col-groups)
                    ps_s03 = pp_row.tile([128, C], f32, tag="sumps", name="ps_s03")
                    nchunks = max(len(s_chunks(c)) for c in range(4))
                    for i in range(nchunks):
                        for c in range(4):
                            ch = s_chunks(c)
                            if i < len(ch):
                                packed_sum(ps_s03, c, [ch[i]], i == 0, i == len(ch) - 1)
                    stage_rows(ps_s03, [(0, TS), (1, TS), (2, TS), (3, TS)])

                    # remaining sum streams: S_4, then B_c/C_c per class,
                    # packed 4 per PSUM tile
                    ps_bc = pp_row.tile([128, C], f32, tag="sumps", name="ps_bc0")
                    packed_sum(ps_bc, 0, s_chunks(4), True, True)
                    pending = [(4, TS)]

                    def bc_flush(force=False):
                        nonlocal ps_bc, pending
                        if pending and (force or len(pending) >= 3):
                            stage_rows(ps_bc, pending)
                            pending = []
                            if not force:
                                ps_bc = pp_row.tile([128, C], f32, tag="sumps",
                                                    name="ps_bc")
                    for c in range(WAY):
                        rows = int(counts[c]) * T
                        wlo = offs[c] // 128
                        nw_c = (rows + 127) // 128
                        # prototypes: P[d, col] accumulated over class rows
                        pt = grp.tile([128, NDCH, C], bf16, tag="pt", bufs=1)
                        p2t = grp.tile([128, NDCH, C], bf16, tag="p2t", bufs=1)
                        for dd in range(NDCH):
                            ps_p = pp_pr.tile([128, C], f32, tag="prps")
                            for wi in range(nw_c):
                                nc.tensor.matmul(ps_p, s_v[:, wlo + wi, dd * 128:(dd + 1) * 128],
                                                 exp_t[:, wlo + wi],
                                                 start=(wi == 0), stop=(wi == nw_c - 1))
                            if dd % 2 == 0:
                                nc.scalar.activation(pt[:, dd], ps_p, AF.Copy)
                            else:
                                nc.vector.tensor_copy(pt[:, dd], ps_p)
                        nc.scalar.activation(p2t, pt, AF.Square)
                        nc.vector.tensor_mul(pt, pt, qv3)  # now <q_v, P> terms
                        # interleave B_c / C_c chunk streams for PE concurrency
                        sb, sc = len(pending), len(pending) + 1
                        for k in range(NDCH):
                            packed_sum(ps_bc, sb, [(pt[:, k], 128)], k == 0, k == NDCH - 1)
                            packed_sum(ps_bc, sc, [(p2t[:, k], 128)], k == 0, k == NDCH - 1)
                        pending += [(c, TB), (c, TC)]
                        bc_flush()
                    bc_flush(force=True)

                    # dist = A - 2 B/S + C/S^2 ; logits = -sum_a dist / T
                    sinv = rows_pool.tile([WAY, C], f32, tag="sinv", bufs=1)
                    nc.vector.reciprocal(sinv, packed[:, TS])
                    u = rows_pool.tile([WAY, C], f32, tag="u", bufs=1)
                    nc.vector.tensor_mul(u, packed[:, TC, :], sinv)
                    nc.vector.scalar_tensor_tensor(u, packed[:, TB, :], -2.0, u,
                                                   ALU.mult, ALU.add)
                    nc.vector.tensor_mul(u, u, sinv)
                    nc.vector.tensor_add(u, u, a5)
                    u4 = u.rearrange("w (q a) -> w q a", a=T)
                    red = rows_pool.tile([WAY, G_Q], f32, tag="red", bufs=1)
                    nc.vector.reduce_sum(red, u4, mybir.AxisListType.X)
                    nc.scalar.activation(logits5[:, g * G_Q:(g + 1) * G_Q], red,
                                         AF.Copy, scale=-1.0 / T)

            nc.sync.dma_start(out_d.rearrange("q c -> c q"), logits5)

    nc.compile()
    return nc


def kernel(support_set, support_labels, queries, k_w, k_b, v_w, v_b, ln_g, ln_b):
    import concourse.bass_utils as bass_utils

    support_set = np.asarray(support_set, dtype=np.float32)
    queries = np.asarray(queries, dtype=np.float32)
    labels = np.asarray(support_labels, dtype=np.int32)
    k_w = np.asarray(k_w, dtype=np.float32)
    v_w = np.asarray(v_w, dtype=np.float32)
    k_b = np.asarray(k_b, dtype=np.float32)
    v_b = np.asarray(v_b, dtype=np.float32)
    ln_g = np.asarray(ln_g, dtype=np.float32)
    ln_b = np.asarray(ln_b, dtype=np.float32)

    pe = _pos_encoding()
    s = support_set + pe[None]
    q = queries + pe[None]
    order = np.argsort(labels, kind="stable")
    counts = np.bincount(labels, minlength=WAY)
    s_sorted = s[order]
    trivial_gb = bool(np.all(ln_g == 1.0) and np.all(ln_b == 0.0))

    key = (tuple(int(x) for x in counts), trivial_gb)
    if key not in _CACHE:
        _CACHE[key] = _build_kernel(counts, trivial_gb)
    nc = _CACHE[key]

    W = np.zeros((KPAD, 6 * OUT_DIM), np.float32)
    for j in range(TSS):
        W[:IN_DIM, j * OUT_DIM:(j + 1) * OUT_DIM] = k_w[j * IN_DIM:(j + 1) * IN_DIM]
        W[:IN_DIM, (TSS + j) * OUT_DIM:(TSS + j + 1) * OUT_DIM] = v_w[j * IN_DIM:(j + 1) * IN_DIM]
        W[IN_DIM, j * OUT_DIM:(j + 1) * OUT_DIM] = k_b / TSS
        W[IN_DIM, (TSS + j) * OUT_DIM:(TSS + j + 1) * OUT_DIM] = v_b / TSS
    w_perm = np.ascontiguousarray(
        W.reshape(NKCH, 128, NMB, 128).transpose(1, 2, 0, 3)).astype(BF16)
    g_in = np.ascontiguousarray(ln_g.reshape(NDCH, 128).T).astype(BF16)
    b_in = np.ascontiguousarray(ln_b.reshape(NDCH, 128).T).astype(BF16)

    in_maps = []
    for core in range(N_CORES):
        qs = q[core * NQL:(core + 1) * NQL]
        X = np.concatenate([s_sorted.reshape(-1, IN_DIM), qs.reshape(-1, IN_DIM)], 0)
        XT = np.zeros((KPAD, NX), np.float32)
        XT[:IN_DIM] = X.T
        XT[IN_DIM] = 1.0
        x_perm = np.ascontiguousarray(
            XT.reshape(NKCH, 128, NX).transpose(1, 0, 2)).astype(BF16)
        in_maps.append({"x": x_perm, "w": w_perm, "lng": g_in, "lnb": b_in})

    global _LAST_IN_MAPS
    _LAST_IN_MAPS = in_maps
    res = bass_utils.run_bass_kernel_spmd(nc, in_maps, core_ids=list(range(N_CORES)))
    return np.concatenate([res.results[i]["out"] for i in range(N_CORES)], 0)


_LAST_IN_MAPS = None


# revision 31
# speedup vs baseline: 1.2727x; 1.0274x over previous
"""Trainium2 Bass kernel for the CNN-TRX few-shot attention head.

Sharding: data-parallel over the 200 queries (25 per NeuronCore); support set
and weights replicated per core. All matmuls in bf16 with fp32 PSUM:

  1. Frame projection in transposed layout: f_T[d, frame] for all 6 weight
     blocks (k_w/v_w x 3 tuple positions); biases folded via an augmented
     ones-row of X.
  2. Tuple gather (C(8,3)=56 frame triples) as 2-stage DVE column adds.
  3. LayerNorm of K projections column-wise: stats via ones-matmuls, Rsqrt on
     ACT, gpsimd partition-broadcast, two DVE passes.
  4. scoresT = s_k_pad^T q_k with supports sorted by class and class blocks
     padded to 128 rows; exp via ACT (no max-subtract: LN'd scores are O(1),
     Cauchy-Schwarz bounds |score| <= 34 so exp stays finite in fp32).
  5. Per-class prototypes in T-layout; distance terms ||q_v||^2, <q_v,P>,
     ||P||^2, sum(exp) via ones-matmul column reductions; final combine on
     single-partition rows; logits = -sum_a dist / 56.
"""

import math
from itertools import combinations

import ml_dtypes
import numpy as np

SEQ = 8
IN_DIM = 2048
OUT_DIM = 1152
TSS = 3
WAY = 5
N_SUPPORT = 25
N_QUERIES = 200
PE_SCALE = 0.1
LN_EPS = 1e-5
T = 56
N_CORES = 8
NQL = N_QUERIES // N_CORES      # queries per core
G_Q = 5                         # queries per inner group
N_GROUPS = NQL // G_Q
C = G_Q * T                     # score columns per group (280)
KPAD = 2176                     # 17 * 128 (2048 data + ones row + zero pad)
NKCH = KPAD // 128
NDCH = OUT_DIM // 128           # 9
NMB = 6 * OUT_DIM // 128        # 54 projection column blocks
NX = SEQ * 2 * N_SUPPORT        # 400 frame columns per core
PAIRS = [(t0, t1) for t0 in range(SEQ - 2) for t1 in range(t0 + 1, SEQ - 1)]
LN_CHUNK = 448                  # LayerNorm column chunk (PSUM free-dim <= 512)
BF16 = ml_dtypes.bfloat16

_CACHE = {}


def _pos_encoding():
    pos = np.arange(SEQ, dtype=np.float32)[:, None]
    div = np.exp(np.arange(0, IN_DIM, 2, dtype=np.float32) * -(math.log(10000.0) / IN_DIM))
    pe = np.zeros((SEQ, IN_DIM), dtype=np.float32)
    pe[:, 0::2] = np.sin(pos * div) * PE_SCALE
    pe[:, 1::2] = np.cos(pos * div) * PE_SCALE
    return pe


def _class_layout(counts):
    offs, off = [], 0
    for c in range(WAY):
        offs.append(off)
        off += ((counts[c] * T + 127) // 128) * 128
    return offs, off


def _build_kernel(counts, trivial_gb):
    import concourse.mybir as mybir
    import concourse.tile as tile
    from concourse import bacc
    from concourse.masks import make_identity

    f32 = mybir.dt.float32
    bf16 = mybir.dt.bfloat16
    AF = mybir.ActivationFunctionType
    ALU = mybir.AluOpType
    offs, nb_pad = _class_layout(counts)
    nwch = nb_pad // 128
    inv_sqrt = 1.0 / math.sqrt(OUT_DIM)

    nc = bacc.Bacc("TRN2", target_bir_lowering=False, debug=False,
                   enable_asserts=False, num_devices=N_CORES)

    x_d = nc.dram_tensor("x", [128, NKCH, NX], bf16, kind="ExternalInput").ap()
    w_d = nc.dram_tensor("w", [128, NMB, NKCH, 128], bf16, kind="ExternalInput").ap()
    g_d = nc.dram_tensor("lng", [128, NDCH], bf16, kind="ExternalInput").ap()
    b_d = nc.dram_tensor("lnb", [128, NDCH], bf16, kind="ExternalInput").ap()
    out_d = nc.dram_tensor("out", [NQL, WAY], f32, kind="ExternalOutput").ap()

    with tile.TileContext(nc) as tc:
        with tc.tile_pool(name="big", bufs=1) as big, \
             tc.tile_pool(name="small", bufs=1) as small:
            # frame projections, T-layout; one tile per weight block so
            # consumers only wait for the blocks they read (Tile tracks
            # dependencies per tile, not per region)
            f_b = [big.tile([128, NDCH, NX], bf16, name=f"f_b{j}") for j in range(6)]
            s_kT = big.tile([128, NDCH, nb_pad], bf16)      # LN'd support K, padded cols
            s_v = big.tile([128, nwch, OUT_DIM], bf16)      # support V, row-natural padded
            ones_sb = small.tile([128, 1], bf16)
            nc.vector.memset(ones_sb, 1.0)
            eps_sb = small.tile([1, 1], f32)
            nc.vector.memset(eps_sb, LN_EPS)
            g_sb = small.tile([128, NDCH], bf16)
            b_sb = small.tile([128, NDCH], bf16)
            nc.sync.dma_start(g_sb, g_d)
            nc.sync.dma_start(b_sb, b_d)
            logits5 = small.tile([WAY, NQL], f32)

            # ---------- Phase 1: frame projections ----------
            # sprep/pp_t open BEFORE the phase-1 pools so their SBUF/PSUM space
            # does not alias xt/xw: otherwise the support-side gathers inherit
            # address-level WAW deps on the last projection and cannot overlap.
            sprep_cm = tc.tile_pool(name="sprep", bufs=2)
            pp_t_cm = tc.tile_pool(name="pp_t", bufs=4, space="PSUM")
            sprep = sprep_cm.__enter__()
            pp_t = pp_t_cm.__enter__()
            with tc.tile_pool(name="xt_pool", bufs=1) as xt_pool, \
                 tc.tile_pool(name="xw", bufs=3) as xw, \
                 tc.tile_pool(name="pp_proj", bufs=4, space="PSUM") as pp_proj:
                xt = xt_pool.tile([128, NKCH, NX], bf16)
                nc.sync.dma_start(xt, x_d)
                for m in range(NMB):
                    wm = xw.tile([128, NKCH, 128], bf16, tag="wslab")
                    nc.sync.dma_start(wm, w_d[:, m])
                    ps = pp_proj.tile([128, NX], f32, tag="projps")
                    for k in range(NKCH):
                        nc.tensor.matmul(ps, wm[:, k], xt[:, k],
                                         start=(k == 0), stop=(k == NKCH - 1))
                    nc.scalar.activation(f_b[m // NDCH][:, m % NDCH], ps, AF.Copy)

            f_i = [fb.rearrange("p d (i s) -> p d i s", s=SEQ) for fb in f_b]

            def gather_one(dst4, kv, items0, n_items, pool):
                """dst4 [128, NDCH, n_items, T] = tuple-gathered frame
                projections for one path (kv=0: K blocks 0-2, kv=1: V 3-5)."""
                isl = slice(items0, items0 + n_items)
                b0, b1, b2 = (f_i[3 * kv + j] for j in range(TSS))
                p2 = pool.tile([128, NDCH, n_items, len(PAIRS)], bf16,
                               tag=f"pairs{kv}", name="p2")
                pi = 0
                for t0 in range(SEQ - 2):
                    run = SEQ - 2 - t0
                    a = b0[:, :, isl, t0:t0 + 1]
                    b = b1[:, :, isl, t0 + 1:t0 + 1 + run]
                    nc.vector.tensor_add(p2[:, :, :, pi:pi + run],
                                         a.to_broadcast(b.shape), b)
                    pi += run
                ai = 0
                for pi, (t0, t1) in enumerate(PAIRS):
                    run = SEQ - 1 - t1
                    a = p2[:, :, :, pi:pi + 1]
                    b = b2[:, :, isl, t1 + 1:t1 + 1 + run]
                    nc.vector.tensor_add(dst4[:, :, :, ai:ai + run],
                                         a.to_broadcast(b.shape), b)
                    ai += run

            def col_ln(raw, cols, pool, psum_pool, out=None):
                """Column-wise LayerNorm of raw [128, NDCH, cols] (T-layout);
                in place unless `out` is given."""
                if out is None:
                    out = raw
                for c0 in range(0, cols, LN_CHUNK):
                    cw = min(LN_CHUNK, cols - c0)
                    r = raw[:, :, c0:c0 + cw]
                    o = out[:, :, c0:c0 + cw]
                    sq = pool.tile([128, NDCH, cw], bf16, tag="lnsq", name="lnsq", bufs=1)
                    nc.scalar.activation(sq, r, AF.Square)
                    ps_s = psum_pool.tile([1, cw], f32, tag="lnps", name="lnps")
                    ps_q = psum_pool.tile([1, cw], f32, tag="lnps", name="lnps")
                    for k in range(NDCH):
                        nc.tensor.matmul(ps_s, ones_sb, r[:, k],
                                         start=(k == 0), stop=(k == NDCH - 1))
                    for k in range(NDCH):
                        nc.tensor.matmul(ps_q, ones_sb, sq[:, k],
                                         start=(k == 0), stop=(k == NDCH - 1))
                    m_r = pool.tile([1, cw], f32, tag="lnm", name="lnm")
                    v_r = pool.tile([1, cw], f32, tag="lnv", name="lnv")
                    mm = pool.tile([1, cw], f32, tag="lnmm", name="lnmm")
                    nc.scalar.activation(m_r, ps_s, AF.Copy, scale=1.0 / OUT_DIM)
                    nc.scalar.activation(v_r, ps_q, AF.Copy, scale=1.0 / OUT_DIM)
                    nc.vector.tensor_mul(mm, m_r, m_r)
                    nc.vector.tensor_sub(v_r, v_r, mm)
                    nc.scalar.activation(v_r, v_r, AF.Sqrt, bias=eps_sb)
                    nc.vector.reciprocal(v_r, v_r)
                    m_b = pool.tile([128, cw], f32, tag="lnmb", name="lnmb", bufs=1)
                    a_b = pool.tile([128, cw], f32, tag="lnab", name="lnab", bufs=1)
                    nc.gpsimd.partition_broadcast(m_b, m_r)
                    nc.gpsimd.partition_broadcast(a_b, v_r)
                    mb3 = m_b[:, None, :].to_broadcast([128, NDCH, cw])
                    ab3 = a_b[:, None, :].to_broadcast([128, NDCH, cw])
                    nc.vector.tensor_sub(r, r, mb3)
                    nc.vector.tensor_mul(o, r, ab3)
                    if not trivial_gb:
                        for k in range(NDCH):
                            nc.vector.tensor_scalar(o[:, k], o[:, k],
                                                    g_sb[:, k:k + 1], b_sb[:, k:k + 1],
                                                    ALU.mult, ALU.add)

            # ---------- Phase 2: support-side tensors ----------
            # sprep coexists with the phase-1 pools (opened in the same scope,
            # before phase-1 pools closed above would reuse its space) so the
            # gathers run on the idle DVE while projections stream on the PE.
            # K path gathers straight into the persistent s_kT (LN in place);
            # V path gathers into a small per-class scratch, transposed
            # class-by-class into s_v.
            if True:
                ident = small.tile([128, 128], bf16)
                make_identity(nc, ident)
                max_ch = max((int(counts[c]) * T + 127) // 128 for c in range(WAY))
                start_item = 0
                for c in range(WAY):
                    n_c = int(counts[c])
                    rows = n_c * T
                    nch = (rows + 127) // 128
                    wlo = offs[c] // 128
                    pad_lo = offs[c] + rows
                    pad_hi = offs[c + 1] if c + 1 < WAY else nb_pad
                    if pad_hi > pad_lo:
                        nc.gpsimd.memset(s_kT[:, :, pad_lo:pad_hi], 0.0)
                    dst_k = s_kT[:, :, offs[c]:offs[c] + rows].rearrange(
                        "p m (n a) -> p m n a", a=T)
                    gather_one(dst_k, 0, start_item, n_c, sprep)
                    s_vT_c = sprep.tile([128, NDCH, max_ch * 128], bf16,
                                        tag="svtc", name="svtc")
                    if rows < nch * 128:
                        nc.gpsimd.memset(s_vT_c[:, :, rows:nch * 128], 0.0)
                    dst_v = s_vT_c[:, :, :rows].rearrange("p m (n a) -> p m n a", a=T)
                    gather_one(dst_v, 1, start_item, n_c, sprep)
                    for w in range(nch):
                        for dd in range(NDCH):
                            ps = pp_t.tile([128, 128], bf16, tag="tps")
                            nc.tensor.transpose(ps, s_vT_c[:, dd, w * 128:(w + 1) * 128],
                                                ident)
                            if dd % 2 == 0:
                                nc.vector.tensor_copy(
                                    s_v[:, wlo + w, dd * 128:(dd + 1) * 128], ps)
                            else:
                                nc.scalar.activation(
                                    s_v[:, wlo + w, dd * 128:(dd + 1) * 128], ps, AF.Copy)
                    start_item += n_c
                # LayerNorm all support columns at once, in place (pad columns
                # are zero and stay zero)
                with tc.tile_pool(name="pp_s", bufs=2, space="PSUM") as pp_s:
                    col_ln(s_kT, nb_pad, sprep, pp_s)
            pp_t_cm.__exit__(None, None, None)
            sprep_cm.__exit__(None, None, None)

            # ---------- Phase 3: per-group query pipeline ----------
            # Column sums (S_c, B_c, C_c, A) are M=1 ones-matmuls packed 4 per
            # PSUM tile at partitions {0,32,64,96} via tile_position so the PE
            # runs them concurrently in distinct 32-column groups. The rows are
            # then DMA-packed into a [WAY, 4, C] tile (partition = class) so
            # the final combine runs on 5 lanes instead of 1.
            TS, TB, TC, TA = 0, 1, 2, 3  # term slots in the packed tile

            def packed_sum(ps_tile, slot, rhs_chunks, first, last):
                """Accumulate sum-over-partitions of each rhs chunk into
                ps_tile[32*slot] using a col-group tile_position."""
                out = ps_tile[32 * slot:32 * slot + 1]
                for i, (rhs, kc) in enumerate(rhs_chunks):
                    nc.tensor.matmul(out, ones_sb[:kc], rhs, start=(first and i == 0),
                                     stop=(last and i == len(rhs_chunks) - 1),
                                     tile_position=(0, 32 * slot),
                                     skip_group_check=True)

            with tc.tile_pool(name="grp", bufs=2) as grp, \
                 tc.tile_pool(name="rows", bufs=2) as rows_pool, \
                 tc.tile_pool(name="pp_sc", bufs=2, space="PSUM") as pp_sc, \
                 tc.tile_pool(name="pp_pr", bufs=2, space="PSUM") as pp_pr, \
                 tc.tile_pool(name="pp_row", bufs=2, space="PSUM") as pp_row:
                for g in range(N_GROUPS):
                    q_kT = grp.tile([128, NDCH, G_Q, T], bf16, tag="qk")
                    q_vT = grp.tile([128, NDCH, G_Q, T], bf16, tag="qv")
                    items0 = N_SUPPORT + g * G_Q
                    gather_one(q_kT, 0, items0, G_Q, grp)
                    gather_one(q_vT, 1, items0, G_Q, grp)
                    qk3 = q_kT.rearrange("p m q a -> p m (q a)")
                    qv3 = q_vT.rearrange("p m q a -> p m (q a)")
                    col_ln(qk3, C, grp, pp_row)

                    # scoresT + exp
                    exp_t = grp.tile([128, nwch, C], bf16, tag="exp")
                    for w in range(nwch):
                        ps = pp_sc.tile([128, C], f32, tag="scps")
                        for k in range(NDCH):
                            nc.tensor.matmul(ps, s_kT[:, k, w * 128:(w + 1) * 128],
                                             qk3[:, k], start=(k == 0), stop=(k == NDCH - 1))
                        nc.scalar.activation(exp_t[:, w], ps, AF.Exp, scale=inv_sqrt)

                    packed = rows_pool.tile([WAY, 4, C], f32, tag="packed")

                    # A = ||q_v||^2 per column
                    qsq = grp.tile([128, NDCH, C], bf16, tag="qsq", bufs=1)
                    nc.scalar.activation(qsq, qv3, AF.Square)
                    ps_a = pp_row.tile([128, C], f32, tag="sumps", name="ps_a")
                    packed_sum(ps_a, 0, [(qsq[:, k], 128) for k in range(NDCH)], True, True)
                    a_sb = rows_pool.tile([1, C], f32, tag="a_sb", bufs=1)
                    nc.vector.tensor_copy(a_sb, ps_a[0:1])
                    a5 = rows_pool.tile([WAY, C], f32, tag="a5", bufs=1)
                    nc.gpsimd.partition_broadcast(a5, a_sb)

                    def s_chunks(c):
                        rows = int(counts[c]) * T
                        wlo = offs[c] // 128
                        return [(exp_t[:min(128, rows - wi * 128), wlo + wi],
                                 min(128, rows - wi * 128))
                                for wi in range((rows + 127) // 128)]

                    def stage_rows(ps, rows):
                        """PSUM sum-tile -> SBUF (one lane-parallel copy; engines
                        cannot address partition starts other than 0/32/64/96),
                        then DMA rows {32j} into packed[class, term]."""
                        st = rows_pool.tile([128, C], f32, tag="stage", name="stage")
                        nc.scalar.activation(st, ps, AF.Copy)
                        st4 = st.rearrange("(j z) n -> j z n", z=32)
                        for j, (cc, term) in enumerate(rows):
                            nc.sync.dma_start(packed[cc:cc + 1, term],
                                              st4[j:j + 1, 0, :])

                    # S_0..S_3 packed in one PSUM tile (concurrent col-groups)
                    ps_s03 = pp_row.tile([128, C], f32, tag="sumps", name="ps_s03")
                    nchunks = max(len(s_chunks(c)) for c in range(4))
                    for i in range(nchunks):
                        for c in range(4):
                            ch = s_chunks(c)
                            if i < len(ch):
                                packed_sum(ps_s03, c, [ch[i]], i == 0, i == len(ch) - 1)
                    stage_rows(ps_s03, [(0, TS), (1, TS), (2, TS), (3, TS)])

                    # remaining sum streams: S_4, then B_c/C_c per class,
                    # packed 4 per PSUM tile
                    ps_bc = pp_row.tile([128, C], f32, tag="sumps", name="ps_bc0")
                    packed_sum(ps_bc, 0, s_chunks(4), True, True)
                    pending = [(4, TS)]

                    def bc_flush(force=False):
                        nonlocal ps_bc, pending
                        if pending and (force or len(pending) >= 3):
                            stage_rows(ps_bc, pending)
                            pending = []
                            if not force:
                                ps_bc = pp_row.tile([128, C], f32, tag="sumps",
                                                    name="ps_bc")
                    for c in range(WAY):
                        rows = int(counts[c]) * T
                        wlo = offs[c] // 128
                        nw_c = (rows + 127) // 128
                        # prototypes: P[d, col] accumulated over class rows
                        pt = grp.tile([128, NDCH, C], bf16, tag="pt", bufs=1)
                        p2t = grp.tile([128, NDCH, C], bf16, tag="p2t", bufs=1)
                        for dd in range(NDCH):
                            ps_p = pp_pr.tile([128, C], f32, tag="prps")
                            for wi in range(nw_c):
                                nc.tensor.matmul(ps_p, s_v[:, wlo + wi, dd * 128:(dd + 1) * 128],
                                                 exp_t[:, wlo + wi],
                                                 start=(wi == 0), stop=(wi == nw_c - 1))
                            if dd % 2 == 0:
                                nc.scalar.activation(pt[:, dd], ps_p, AF.Copy)
                            else:
                                nc.vector.tensor_copy(pt[:, dd], ps_p)
                        nc.scalar.activation(p2t, pt, AF.Square)
                        nc.vector.tensor_mul(pt, pt, qv3)  # now <q_v, P> terms
                        # interleave B_c / C_c chunk streams for PE concurrency
                        sb, sc = len(pending), len(pending) + 1
                        for k in range(NDCH):
                            packed_sum(ps_bc, sb, [(pt[:, k], 128)], k == 0, k == NDCH - 1)
                            packed_sum(ps_bc, sc, [(p2t[:, k], 128)], k == 0, k == NDCH - 1)
                        pending += [(c, TB), (c, TC)]
                        bc_flush()
                    bc_flush(force=True)

                    # dist = A - 2 B/S + C/S^2 ; logits = -sum_a dist / T
                    sinv = rows_pool.tile([WAY, C], f32, tag="sinv", bufs=1)
                    nc.vector.reciprocal(sinv, packed[:, TS])
                    u = rows_pool.tile([WAY, C], f32, tag="u", bufs=1)
                    nc.vector.tensor_mul(u, packed[:, TC, :], sinv)
                    nc.vector.scalar_tensor_tensor(u, packed[:, TB, :], -2.0, u,
                                                   ALU.mult, ALU.add)
                    nc.vector.tensor_mul(u, u, sinv)
                    nc.vector.tensor_add(u, u, a5)
                    u4 = u.rearrange("w (q a) -> w q a", a=T)
                    red = rows_pool.tile([WAY, G_Q], f32, tag="red", bufs=1)
                    nc.vector.reduce_sum(red, u4, mybir.AxisListType.X)
                    nc.scalar.activation(logits5[:, g * G_Q:(g + 1) * G_Q], red,
                                         AF.Copy, scale=-1.0 / T)

            nc.sync.dma_start(out_d.rearrange("q c -> c q"), logits5)

    nc.compile()
    return nc


def kernel(support_set, support_labels, queries, k_w, k_b, v_w, v_b, ln_g, ln_b):
    import concourse.bass_utils as bass_utils

    support_set = np.asarray(support_set, dtype=np.float32)
    queries = np.asarray(queries, dtype=np.float32)
    labels = np.asarray(support_labels, dtype=np.int32)
    k_w = np.asarray(k_w, dtype=np.float32)
    v_w = np.asarray(v_w, dtype=np.float32)
    k_b = np.asarray(k_b, dtype=np.float32)
    v_b = np.asarray(v_b, dtype=np.float32)
    ln_g = np.asarray(ln_g, dtype=np.float32)
    ln_b = np.asarray(ln_b, dtype=np.float32)

    pe = _pos_encoding()
    s = support_set + pe[None]
    q = queries + pe[None]
    order = np.argsort(labels, kind="stable")
    counts = np.bincount(labels, minlength=WAY)
    s_sorted = s[order]
    trivial_gb = bool(np.all(ln_g == 1.0) and np.all(ln_b == 0.0))

    key = (tuple(int(x) for x in counts), trivial_gb)
    if key not in _CACHE:
        _CACHE[key] = _build_kernel(counts, trivial_gb)
    nc = _CACHE[key]

    W = np.zeros((KPAD, 6 * OUT_DIM), np.float32)
    for j in range(TSS):
        W[:IN_DIM, j * OUT_DIM:(j + 1) * OUT_DIM] = k_w[j * IN_DIM:(j + 1) * IN_DIM]
        W[:IN_DIM, (TSS + j) * OUT_DIM:(TSS + j + 1) * OUT_DIM] = v_w[j * IN_DIM:(j + 1) * IN_DIM]
        W[IN_DIM, j * OUT_DIM:(j + 1) * OUT_DIM] = k_b / TSS
        W[IN_DIM, (TSS + j) * OUT_DIM:(TSS + j + 1) * OUT_DIM] = v_b / TSS
    w_perm = np.ascontiguousarray(
        W.reshape(NKCH, 128, NMB, 128).transpose(1, 2, 0, 3)).astype(BF16)
    g_in = np.ascontiguousarray(ln_g.reshape(NDCH, 128).T).astype(BF16)
    b_in = np.ascontiguousarray(ln_b.reshape(NDCH, 128).T).astype(BF16)

    in_maps = []
    for core in range(N_CORES):
        qs = q[core * NQL:(core + 1) * NQL]
        X = np.concatenate([s_sorted.reshape(-1, IN_DIM), qs.reshape(-1, IN_DIM)], 0)
        XT = np.zeros((KPAD, NX), np.float32)
        XT[:IN_DIM] = X.T
        XT[IN_DIM] = 1.0
        x_perm = np.ascontiguousarray(
            XT.reshape(NKCH, 128, NX).transpose(1, 0, 2)).astype(BF16)
        in_maps.append({"x": x_perm, "w": w_perm, "lng": g_in, "lnb": b_in})

    global _LAST_IN_MAPS
    _LAST_IN_MAPS = in_maps
    res = bass_utils.run_bass_kernel_spmd(nc, in_maps, core_ids=list(range(N_CORES)))
    return np.concatenate([res.results[i]["out"] for i in range(N_CORES)], 0)


_LAST_IN_MAPS = None
